# revision 45
# baseline (speedup 1.0000x reference)
"""Batched GAT layer (B=8, N=2048, Fin=256, Fout=128) on 8 Trainium2 NeuronCores.

Data-parallel over batch B — one batch element per core. Per core, a
factored-exponential formulation keeps the O(N^2) element-wise work on the
DVE fp16 fast paths and the softmax contraction on the PE with the
denominator fused in as a 129th output column:

  h        = x @ W.T + b                       (PE fp16, fp32 psum)
  s1[i]    = a1.h_i,  s2[j] = a2.h_j           (PE)
  exp(lrelu(s1+s2)) = max(e^{s1}e^{s2}, e^{.4 s1}e^{.4 s2})   (lrelu piecewise)
  u = e^{s1-c1}, u' = e^{.4 s1-c1'}            (ACT, replicated row tiles)
  v = e^{s2-c2}, v' = e^{.4 s2-c2'}            (ACT, per-partition columns)
  p[j,i]   = m[j,i] * max(u_i v_j, u'_i v'_j)  (DVE TS/TT; m = 0/1 mask)
  acc[i,:] = sum_j p[j,i] * [h_j | 1]          (PE, p-stationary, S = col 128)
  out      = elu(acc[:, :128] / acc[:, 128])   (DVE + ACT tail)

Some j-tiles (ALPHA_TILES) instead use the additive-mask path
(em -> Prelu -> Exp on ACT) to balance DVE/ACT load; the host encodes mask
rows per tile type (0/-60 additive vs 1/0 multiplicative, fp16). Shifts
c1,c2 (softmax-invariant) keep everything in fp16 range; calibrated
host-side from cheap score maxima, passed as per-partition bias vectors.

DMA: the mask (8MB fp16) is host-relayouted partition-contiguous in groups
of 4 j-tiles so each of the 4 DMAs moves 16KB-contiguous runs per
partition (big packets, hardware-dynamic queue) instead of 4KB rows.
"""
import numpy as np

B, N, FIN, FOUT = 8, 2048, 256, 128
P = 128
NT = N // P           # 16 n-tiles
NC4 = N // 512        # 4 chunks of 512
NG = 4                # mask DMA groups
GT = NT // NG         # tiles per group (16KB per-partition runs)
ALPHA = 0.4
MASK_NEG = -60.0
MARGIN = 5.2

# j-tiles on the ACT additive-mask path (rest: factored DVE path).
# Spread across DMA groups so both engines stream smoothly.
ALPHA_TILES = frozenset({0, 2, 4, 6, 8, 10, 12, 14})
# f-tiles whose max/mask ops run on the Pool engine (early-mask tiles with
# slack); their matmuls are emitted later in the PE queue
POOL_TILES = (3, 5, 7)

_cache = {}


def _build():
    import concourse.mybir as mybir
    import concourse.tile as tile
    from concourse import bacc
    from concourse.masks import make_identity
    from contextlib import ExitStack

    F32 = mybir.dt.float32
    F16 = mybir.dt.float16
    AF = mybir.ActivationFunctionType
    ALU = mybir.AluOpType

    nc = bacc.Bacc("TRN2", target_bir_lowering=False, debug=False)

    # xT relayouted [128, 2, 2048] partition-contiguous; mask in 4 groups of
    # 4 j-tiles, each [128, 4, 2048] partition-contiguous; out written
    # [128, 16, 128] partition-major (host de-permutes).
    # xT chunked: [p, c*1024 + h*512 + i] = x[c*512 + i, h*128 + p]
    xT_d = nc.dram_tensor("xTp", [P, 2 * N], F16, kind="ExternalInput").ap()
    madj_d = nc.dram_tensor("madjp", [NG * P, GT * N], F16, kind="ExternalInput").ap()
    # all small params packed partition-contiguous into one tensor:
    # [wt0 | wt1 | a1rep | brow-rep | a2col | pad | cvecs-bytes]
    params_d = nc.dram_tensor("params", [P, 640], F16, kind="ExternalInput").ap()
    out_d = nc.dram_tensor("outp", [P, NT * FOUT], F16, kind="ExternalOutput").ap()

    with tile.TileContext(nc) as tc:
        with tc.tile_pool(name="const", bufs=1) as cpool, \
             tc.tile_pool(name="work", bufs=2) as wpool:
            # ---- packed params on the scalar queue; xT chunked on sync ----
            params = cpool.tile([P, 640], F16, tag="params")
            nc.scalar.dma_start(params[:], params_d)
            wt0 = params[:, 0:128]
            wt1 = params[:, 128:256]
            a1rep = params[:, 256:384]
            brow = params[0:1, 384:512]
            a2col = params[:, 512:513]
            cvecs = params[:, 624:640].bitcast(F32)
            # xT whole (8KB runs) first on the sync queue
            xt_all = cpool.tile([P, 2 * N], F16, tag="xt_all")
            nc.sync.dma_start(xt_all[:], xT_d)

            def xt0(c):
                return xt_all[:, c * 512:(c + 1) * 512]

            def xt1(c):
                return xt_all[:, N + c * 512:N + (c + 1) * 512]

            # mask: 4 groups of 4 tiles (16KB per-partition runs — big
            # packets keep the descriptor rate off the critical path),
            # alternating scalar/sync queues in loop-consumption order.
            # gpsimd issues nothing so the Pool engine is free for compute.
            adjm_all = cpool.tile([P, NT * N], F16, tag="adjm_all")
            for g in range(NG):
                eng = nc.scalar if g == 0 else nc.sync
                eng.dma_start(
                    adjm_all[:, g * GT * N:(g + 1) * GT * N],
                    madj_d[g * P:(g + 1) * P, :])

            def adjm(t):
                return adjm_all[:, t * N:(t + 1) * N]

            # constants built on gpsimd
            ident = cpool.tile([P, P], F16, tag="ident")
            make_identity(nc, ident[:])
            ones512 = cpool.tile([1, 512], F16, tag="ones512")
            nc.gpsimd.memset(ones512[:], 1.0)
            zcol = cpool.tile([1, P], F16, tag="zcol")
            nc.gpsimd.memset(zcol[:], 0.0)
            h_aug = []
            for t in range(NT):
                ha = cpool.tile([P, P + 4], F16, tag=f"haug{t}")
                nc.gpsimd.memset(ha[:, P:P + 1], 1.0)
                h_aug.append(ha)

            prep_ctx = ExitStack()
            pst = prep_ctx.enter_context(tc.tile_pool(name="pst", bufs=2, space="PSUM"))

            # ---- hT[o, n] = W x + b (bias via rank-1 matmul) ----
            hT = cpool.tile([FOUT, N], F16, tag="hT")
            for c in range(NC4):
                sl = slice(c * 512, (c + 1) * 512)
                hps = pst.tile([FOUT, 512], F32, tag="hps")
                nc.tensor.matmul(hps[:], wt0[:], xt0(c), start=True, stop=False)
                nc.tensor.matmul(hps[:], wt1[:], xt1(c), start=False, stop=False)
                nc.tensor.matmul(hps[:], brow[:], ones512[:], start=False, stop=True)
                nc.scalar.activation(hT[:, sl], hps[:], AF.Identity)

            # ---- s1b[p, i] = a1 . h_i (replicated across partitions) ----
            s1b = cpool.tile([P, N], F16, tag="s1b")
            for c in range(NC4):
                sl = slice(c * 512, (c + 1) * 512)
                bps = pst.tile([P, 512], F32, tag="bps")
                nc.tensor.matmul(bps[:], a1rep[:], hT[:, sl], start=True, stop=True)
                nc.scalar.activation(s1b[:, sl], bps[:], AF.Identity)

            # ---- s2 columns: s2cols[p, t] = a2 . h_{t*128+p} ----
            s2ps = pst.tile([P, NT], F32, tag="s2ps")
            for t in range(NT):
                nc.tensor.matmul(s2ps[:, t:t + 1], hT[:, t * P:(t + 1) * P],
                                 a2col[:], start=True, stop=True,
                                 skip_group_check=True)
            s2cols = cpool.tile([P, NT], F32, tag="s2cols")
            nc.vector.tensor_copy(s2cols[:], s2ps[:])

            # ---- v / v' columns (fp32) and u / u' reps (fp16) ----
            vcols = cpool.tile([P, NT], F32, tag="vcols")
            nc.scalar.activation(vcols[:], s2cols[:], AF.Exp, bias=cvecs[:, 2:3])
            vpcols = cpool.tile([P, NT], F32, tag="vpcols")
            nc.scalar.activation(vpcols[:], s2cols[:], AF.Exp, bias=cvecs[:, 3:4],
                                 scale=ALPHA)
            urep = cpool.tile([P, N], F16, tag="urep")
            nc.scalar.activation(urep[:], s1b[:], AF.Exp, bias=cvecs[:, 0:1])
            uprep = cpool.tile([P, N], F16, tag="uprep")
            nc.scalar.activation(uprep[:], s1b[:], AF.Exp, bias=cvecs[:, 1:2],
                                 scale=ALPHA)

            # ---- h_aug tiles via PE transpose of hT ----
            for t in range(NT):
                tps = pst.tile([P, P], F16, tag="tps")
                nc.tensor.transpose(tps[:], hT[:, t * P:(t + 1) * P], ident[:])
                nc.vector.tensor_copy(h_aug[t][:, 0:P], tps[:])

            prep_ctx.close()

            # ---- PSUM accumulators: 16 slices packed 3-per-bank ----
            acc_ctx = ExitStack()
            psacc = acc_ctx.enter_context(
                tc.tile_pool(name="psacc", bufs=1, space="PSUM"))
            accb = [psacc.tile([P, 512], F32, tag=f"accb{k}", name=f"accb{k}")
                    for k in range(6)]
            # start+stop zero-fill each bank once; the 16 accumulation slices
            # then run accumulate-only (multiple open start-groups per bank
            # lose their staged first write).
            for k in range(6):
                nc.tensor.matmul(accb[k][:], zcol[:], ones512[:],
                                 start=True, stop=True, skip_group_check=True)

            def acc_ap(q, lo, hi):
                base = (q % 3) * 160
                return accb[q // 3][:, base + lo:base + hi]

            # ---- main loop over j-tiles ----
            def emit_mms(t, pt, last=False):
                for q in range(NT):
                    nc.tensor.matmul(acc_ap(q, 0, 129), pt[:, q * P:(q + 1) * P],
                                     h_aug[t][:, 0:129], start=False, stop=last,
                                     skip_group_check=True)

            # Pool-assisted tiles' matmuls are deferred in the PE queue until
            # the slow Pool ops are certain to have finished (accumulation
            # over j-tiles is order-independent)
            defer_after = {6: 3, 10: 5, 14: 7}
            deferred_pt = {}
            for t in range(NT):
                if t in ALPHA_TILES:
                    em = wpool.tile([P, N], F16, tag="em", bufs=3)
                    nc.vector.tensor_tensor(em[:], adjm(t), s1b[:], ALU.add)
                    lt = wpool.tile([P, N], F16, tag="lt")
                    nc.scalar.activation(lt[:], em[:], AF.Prelu,
                                         bias=s2cols[:, t:t + 1],
                                         scale=1.0, alpha=ALPHA)
                    pt = wpool.tile([P, N], F16, tag="pt", bufs=8)
                    nc.scalar.activation(pt[:], lt[:], AF.Exp, bias=cvecs[:, 4:5])
                else:
                    veng = nc.gpsimd if t in POOL_TILES else nc.vector
                    t1a = wpool.tile([P, N], F16, tag="t1a")
                    nc.vector.tensor_scalar(t1a[:], urep[:], vcols[:, t:t + 1],
                                            None, op0=ALU.mult)
                    t2 = wpool.tile([P, N], F16, tag="t2")
                    nc.vector.tensor_scalar(t2[:], uprep[:], vpcols[:, t:t + 1],
                                            None, op0=ALU.mult)
                    mx = wpool.tile([P, N], F16, tag="lt")
                    nc.vector.tensor_tensor(mx[:], t1a[:], t2[:], ALU.max)
                    pt = wpool.tile([P, N], F16, tag="pt", bufs=8)
                    veng.tensor_tensor(pt[:], mx[:], adjm(t), ALU.mult)

                if t in POOL_TILES:
                    deferred_pt[t] = pt
                else:
                    emit_mms(t, pt, last=(t == NT - 1))
                if t in defer_after:
                    dt = defer_after[t]
                    emit_mms(dt, deferred_pt.pop(dt))

            # ---- tail: normalize + elu, pipelined in halves ----
            hn = cpool.tile([P, N], F16, tag="hn")
            m0 = cpool.tile([P, N], F16, tag="m0")
            ex = cpool.tile([P, N], F16, tag="ex")
            exm1 = cpool.tile([P, N], F16, tag="exm1")
            ov = cpool.tile([P, NT * FOUT], F16, tag="ov")
            for qtr in range(4):
                for q in range(qtr * 4, qtr * 4 + 4):
                    rsq = cpool.tile([P, 1], F32, tag=f"rs{q}")
                    nc.vector.reciprocal(rsq[:], acc_ap(q, 128, 129))
                    if q % 2 == 0:
                        nc.scalar.activation(hn[:, q * P:(q + 1) * P],
                                             acc_ap(q, 0, 128), AF.Identity,
                                             scale=rsq[:])
                    else:
                        nc.vector.tensor_scalar(hn[:, q * P:(q + 1) * P],
                                                acc_ap(q, 0, 128), rsq[:],
                                                None, op0=ALU.mult)
                hs = slice(qtr * 512, qtr * 512 + 512)
                nc.vector.tensor_scalar(m0[:, hs], hn[:, hs], 0.0, None,
                                        op0=ALU.min)
                nc.scalar.activation(ex[:, hs], m0[:, hs], AF.Exp)
                nc.vector.tensor_scalar(exm1[:, hs], ex[:, hs], 1.0, None,
                                        op0=ALU.subtract)
                nc.vector.tensor_tensor(ov[:, hs], exm1[:, hs], hn[:, hs],
                                        ALU.max)
                if qtr % 2 == 1:
                    dhs = slice((qtr - 1) * 512, qtr * 512 + 512)
                    nc.scalar.dma_start(out_d[:, dhs], ov[:, dhs])
            acc_ctx.close()

    nc.compile()
    return nc


def make_in_maps(input, adj, W, b, a):
    x = np.asarray(input, dtype=np.float32)
    adj_np = np.asarray(adj)
    W_np = np.asarray(W, dtype=np.float32)
    b_np = np.asarray(b, dtype=np.float32)
    a_np = np.asarray(a, dtype=np.float32)
    a1 = a_np[:FOUT, 0]
    a2 = a_np[FOUT:, 0]

    # score-range calibration (cheap host dot products, sets fp16 shifts)
    w1 = W_np.T @ a1
    w2 = W_np.T @ a2
    s1 = x @ w1 + float(b_np @ a1)        # [B, N]
    s2 = x @ w2 + float(b_np @ a2)
    c1 = np.float32(s1.max() - MARGIN)
    c2 = np.float32(s2.max() - MARGIN)
    C = np.float32(c1 + c2)
    c1p = np.float32(ALPHA) * c1
    c2p = np.float32(C - c1p)
    cvecs = np.zeros((P, 8), dtype=np.float32)
    cvecs[:, 0] = -c1
    cvecs[:, 1] = -c1p
    cvecs[:, 2] = -c2
    cvecs[:, 3] = -c2p
    cvecs[:, 4] = -C

    # xT partition-contiguous: xTp[p, h*N + i] = x[i, h*128 + p]
    xT = x.transpose(0, 2, 1).reshape(B, 2, P, N)           # [B, h, p, i]
    xTp = np.ascontiguousarray(xT.transpose(0, 2, 1, 3)     # [B, p, h, i]
                               .reshape(B, P, 2 * N)).astype(np.float16)

    # packed small params: [wt0 | wt1 | a1rep | brow-rep | a2col | pad | cvecs]
    wt = W_np.T.astype(np.float16)                          # [256, 128]
    params = np.zeros((P, 640), dtype=np.float16)
    params[:, 0:128] = wt[0:P]
    params[:, 128:256] = wt[P:FIN]
    params[:, 256:384] = np.broadcast_to(a1[:, None], (FOUT, P)).astype(np.float16)
    params[:, 384:512] = np.broadcast_to(b_np[None, :], (P, FOUT)).astype(np.float16)
    params[:, 512] = a2.astype(np.float16)
    params[:, 624:640] = cvecs.view(np.float16)

    # mask [B, j, i] per-tile-type encoding, then partition-contiguous
    # groups: madjp[g*128 + p, (t%4)*N + i] = enc(adj[i, (4g + t%4)*128 + p])
    adjT = adj_np.transpose(0, 2, 1)       # [B, j, i]
    madj = np.empty((B, N, N), dtype=np.float16)
    alpha_rows = np.zeros(N, dtype=bool)
    for t in ALPHA_TILES:
        alpha_rows[t * P:(t + 1) * P] = True
    madj[:, alpha_rows, :] = np.where(
        adjT[:, alpha_rows, :] > 0, 0.0, MASK_NEG).astype(np.float16)
    madj[:, ~alpha_rows, :] = (adjT[:, ~alpha_rows, :] > 0).astype(np.float16)
    mg = madj.reshape(B, NG, GT, P, N)                       # [B, g, tg, p, i]
    madjp = np.ascontiguousarray(mg.transpose(0, 1, 3, 2, 4)  # [B, g, p, tg, i]
                                 .reshape(B, NG * P, GT * N))

    return [{"xTp": xTp[c], "madjp": madjp[c], "params": params}
            for c in range(B)]


def kernel(input, adj, W, b, a):
    from concourse.bass_utils import run_bass_kernel_spmd

    if "nc" not in _cache:
        _cache["nc"] = _build()
    nc = _cache["nc"]

    in_maps = make_in_maps(input, adj, W, b, a)
    res = run_bass_kernel_spmd(nc, in_maps, core_ids=list(range(B)))
    # outp[p, q*128 + o] -> out[q*128 + p, o]
    out = np.stack([
        np.asarray(res.results[c]["outp"]).reshape(P, NT, FOUT)
        .transpose(1, 0, 2).reshape(N, FOUT)
        for c in range(B)
    ])
    return np.ascontiguousarray(out.astype(np.float32))


# revision 48
# speedup vs baseline: 1.1603x; 1.1603x over previous
"""Batched GAT layer (B=8, N=2048, Fin=256, Fout=128) on 8 Trainium2 NeuronCores.

Data-parallel over batch B — one batch element per core. Per core, a
factored-exponential formulation keeps the O(N^2) element-wise work on the
DVE fp16 fast paths and the softmax contraction on the PE with the
denominator fused in as a 129th output column:

  h        = x @ W.T + b                       (PE fp16, fp32 psum)
  s1[i]    = a1.h_i,  s2[j] = a2.h_j           (PE)
  exp(lrelu(s1+s2)) = max(e^{s1}e^{s2}, e^{.4 s1}e^{.4 s2})   (lrelu piecewise)
  u = e^{s1-c1}, u' = e^{.4 s1-c1'}            (ACT, replicated row tiles)
  v = e^{s2-c2}, v' = e^{.4 s2-c2'}            (ACT, per-partition columns)
  p[j,i]   = m[j,i] * max(u_i v_j, u'_i v'_j)  (DVE TS/TT; m = 0/1 mask)
  acc[i,:] = sum_j p[j,i] * [h_j | 1]          (PE, p-stationary, S = col 128)
  out      = elu(acc[:, :128] / acc[:, 128])   (DVE + ACT tail)

Some j-tiles (ALPHA_TILES) instead use the additive-mask path
(em -> Prelu -> Exp on ACT) to balance DVE/ACT load; the host encodes mask
rows per tile type (0/-60 additive vs 1/0 multiplicative, fp16). Shifts
c1,c2 (softmax-invariant) keep everything in fp16 range; calibrated
host-side from cheap score maxima, passed as per-partition bias vectors.

DMA: the mask (8MB fp16) is host-relayouted partition-contiguous in groups
of 4 j-tiles so each of the 4 DMAs moves 16KB-contiguous runs per
partition (big packets, hardware-dynamic queue) instead of 4KB rows.
"""
import numpy as np

B, N, FIN, FOUT = 8, 2048, 256, 128
P = 128
NT = N // P           # 16 n-tiles
NC4 = N // 512        # 4 chunks of 512
NG = 4                # mask DMA groups
GT = NT // NG         # tiles per group (16KB per-partition runs)
ALPHA = 0.4
MASK_NEG = -60.0
MARGIN = 5.2

# j-tiles on the ACT additive-mask path (rest: factored DVE path).
# Spread across DMA groups so both engines stream smoothly.
ALPHA_TILES = frozenset({0, 2, 4, 6, 8, 10, 12, 14})
# f-tiles whose max/mask ops run on the Pool engine (early-mask tiles with
# slack); their matmuls are emitted later in the PE queue
POOL_TILES = (3, 5, 7)

_cache = {}


def _build():
    import concourse.mybir as mybir
    import concourse.tile as tile
    from concourse import bacc
    from concourse.masks import make_identity
    from contextlib import ExitStack

    F32 = mybir.dt.float32
    F16 = mybir.dt.float16
    AF = mybir.ActivationFunctionType
    ALU = mybir.AluOpType

    nc = bacc.Bacc("TRN2", target_bir_lowering=False, debug=False)

    # xT relayouted [128, 2, 2048] partition-contiguous; mask in 4 groups of
    # 4 j-tiles, each [128, 4, 2048] partition-contiguous; out written
    # [128, 16, 128] partition-major (host de-permutes).
    # xT chunked: [p, c*1024 + h*512 + i] = x[c*512 + i, h*128 + p]
    xT_d = nc.dram_tensor("xTp", [P, 2 * N], F16, kind="ExternalInput").ap()
    madj_d = nc.dram_tensor("madjp", [NG * P, GT * N], F16, kind="ExternalInput").ap()
    # all small params packed partition-contiguous into one tensor:
    # [wt0 | wt1 | a1rep | brow-rep | a2col | pad | cvecs-bytes]
    params_d = nc.dram_tensor("params", [P, 640], F16, kind="ExternalInput").ap()
    out_d = nc.dram_tensor("outp", [P, NT * FOUT], F16, kind="ExternalOutput").ap()

    with tile.TileContext(nc) as tc:
        with tc.tile_pool(name="const", bufs=1) as cpool, \
             tc.tile_pool(name="work", bufs=2) as wpool:
            # ---- packed params on the scalar queue; xT chunked on sync ----
            params = cpool.tile([P, 640], F16, tag="params")
            nc.scalar.dma_start(params[:], params_d)
            wt0 = params[:, 0:128]
            wt1 = params[:, 128:256]
            a1rep = params[:, 256:384]
            brow = params[0:1, 384:512]
            a2col = params[:, 512:513]
            cvecs = params[:, 624:640].bitcast(F32)
            # xT whole (8KB runs) first on the sync queue
            xt_all = cpool.tile([P, 2 * N], F16, tag="xt_all")
            nc.sync.dma_start(xt_all[:], xT_d)

            def xt0(c):
                return xt_all[:, c * 512:(c + 1) * 512]

            def xt1(c):
                return xt_all[:, N + c * 512:N + (c + 1) * 512]

            # mask: 4 groups of 4 tiles (16KB per-partition runs — big
            # packets keep the descriptor rate off the critical path),
            # alternating scalar/sync queues in loop-consumption order.
            # gpsimd issues nothing so the Pool engine is free for compute.
            adjm_all = cpool.tile([P, NT * N], F16, tag="adjm_all")
            for g in range(NG):
                eng = nc.scalar if g == 0 else nc.sync
                eng.dma_start(
                    adjm_all[:, g * GT * N:(g + 1) * GT * N],
                    madj_d[g * P:(g + 1) * P, :])

            def adjm(t):
                return adjm_all[:, t * N:(t + 1) * N]

            # constants built on gpsimd
            ident = cpool.tile([P, P], F16, tag="ident")
            make_identity(nc, ident[:])
            ones512 = cpool.tile([1, 512], F16, tag="ones512")
            nc.gpsimd.memset(ones512[:], 1.0)
            zcol = cpool.tile([1, P], F16, tag="zcol")
            nc.gpsimd.memset(zcol[:], 0.0)
            h_aug = []
            for t in range(NT):
                ha = cpool.tile([P, P + 4], F16, tag=f"haug{t}")
                nc.gpsimd.memset(ha[:, P:P + 1], 1.0)
                h_aug.append(ha)

            prep_ctx = ExitStack()
            pst = prep_ctx.enter_context(tc.tile_pool(name="pst", bufs=2, space="PSUM"))

            # ---- hT[o, n] = W x + b (bias via rank-1 matmul) ----
            hT = cpool.tile([FOUT, N], F16, tag="hT")
            for c in range(NC4):
                sl = slice(c * 512, (c + 1) * 512)
                hps = pst.tile([FOUT, 512], F32, tag="hps")
                nc.tensor.matmul(hps[:], wt0[:], xt0(c), start=True, stop=False)
                nc.tensor.matmul(hps[:], wt1[:], xt1(c), start=False, stop=False)
                nc.tensor.matmul(hps[:], brow[:], ones512[:], start=False, stop=True)
                nc.scalar.activation(hT[:, sl], hps[:], AF.Identity)

            # ---- s1b[p, i] = a1 . h_i (replicated across partitions) ----
            s1b = cpool.tile([P, N], F16, tag="s1b")
            for c in range(NC4):
                sl = slice(c * 512, (c + 1) * 512)
                bps = pst.tile([P, 512], F32, tag="bps")
                nc.tensor.matmul(bps[:], a1rep[:], hT[:, sl], start=True, stop=True)
                nc.scalar.activation(s1b[:, sl], bps[:], AF.Identity)

            # ---- s2 columns: s2cols[p, t] = a2 . h_{t*128+p} ----
            s2ps = pst.tile([P, NT], F32, tag="s2ps")
            for t in range(NT):
                nc.tensor.matmul(s2ps[:, t:t + 1], hT[:, t * P:(t + 1) * P],
                                 a2col[:], start=True, stop=True,
                                 skip_group_check=True)
            s2cols = cpool.tile([P, NT], F32, tag="s2cols")
            nc.vector.tensor_copy(s2cols[:], s2ps[:])

            # ---- v / v' columns (fp32) and u / u' reps (fp16) ----
            vcols = cpool.tile([P, NT], F32, tag="vcols")
            nc.scalar.activation(vcols[:], s2cols[:], AF.Exp, bias=cvecs[:, 2:3])
            vpcols = cpool.tile([P, NT], F32, tag="vpcols")
            nc.scalar.activation(vpcols[:], s2cols[:], AF.Exp, bias=cvecs[:, 3:4],
                                 scale=ALPHA)
            urep = cpool.tile([P, N], F16, tag="urep")
            nc.scalar.activation(urep[:], s1b[:], AF.Exp, bias=cvecs[:, 0:1])
            uprep = cpool.tile([P, N], F16, tag="uprep")
            nc.scalar.activation(uprep[:], s1b[:], AF.Exp, bias=cvecs[:, 1:2],
                                 scale=ALPHA)

            # ---- h_aug tiles via PE transpose of hT ----
            for t in range(NT):
                tps = pst.tile([P, P], F16, tag="tps")
                nc.tensor.transpose(tps[:], hT[:, t * P:(t + 1) * P], ident[:])
                nc.vector.tensor_copy(h_aug[t][:, 0:P], tps[:])

            prep_ctx.close()

            # ---- PSUM accumulators: 16 slices packed 3-per-bank ----
            acc_ctx = ExitStack()
            psacc = acc_ctx.enter_context(
                tc.tile_pool(name="psacc", bufs=1, space="PSUM"))
            accb = [psacc.tile([P, 512], F32, tag=f"accb{k}", name=f"accb{k}")
                    for k in range(6)]
            # start+stop zero-fill each bank once; the 16 accumulation slices
            # then run accumulate-only (multiple open start-groups per bank
            # lose their staged first write).
            for k in range(6):
                nc.tensor.matmul(accb[k][:], zcol[:], ones512[:],
                                 start=True, stop=True, skip_group_check=True)

            def acc_ap(q, lo, hi):
                base = (q % 3) * 160
                return accb[q // 3][:, base + lo:base + hi]

            # ---- main loop over j-tiles ----
            def emit_mms(t, pt, last=False):
                for q in range(NT):
                    nc.tensor.matmul(acc_ap(q, 0, 129), pt[:, q * P:(q + 1) * P],
                                     h_aug[t][:, 0:129], start=False, stop=last,
                                     skip_group_check=True)

            defer_after = {}
            deferred_pt = {}
            for t in range(NT):
                if t in ALPHA_TILES:
                    em = wpool.tile([P, N], F16, tag="em", bufs=3)
                    nc.vector.tensor_tensor(em[:], adjm(t), s1b[:], ALU.add)
                    lt = wpool.tile([P, N], F16, tag="lt")
                    nc.scalar.activation(lt[:], em[:], AF.Prelu,
                                         bias=s2cols[:, t:t + 1],
                                         scale=1.0, alpha=ALPHA)
                    pt = wpool.tile([P, N], F16, tag="pt", bufs=8)
                    nc.scalar.activation(pt[:], lt[:], AF.Exp, bias=cvecs[:, 4:5])
                else:
                    veng = nc.vector
                    t1a = wpool.tile([P, N], F16, tag="t1a")
                    nc.vector.tensor_scalar(t1a[:], urep[:], vcols[:, t:t + 1],
                                            None, op0=ALU.mult)
                    t2 = wpool.tile([P, N], F16, tag="t2")
                    nc.vector.tensor_scalar(t2[:], uprep[:], vpcols[:, t:t + 1],
                                            None, op0=ALU.mult)
                    mx = wpool.tile([P, N], F16, tag="lt")
                    nc.vector.tensor_tensor(mx[:], t1a[:], t2[:], ALU.max)
                    pt = wpool.tile([P, N], F16, tag="pt", bufs=8)
                    veng.tensor_tensor(pt[:], mx[:], adjm(t), ALU.mult)

                if defer_after and t in POOL_TILES:
                    deferred_pt[t] = pt
                else:
                    emit_mms(t, pt, last=(t == NT - 1))
                if t in defer_after:
                    dt = defer_after[t]
                    emit_mms(dt, deferred_pt.pop(dt))

            # ---- tail: normalize + elu, pipelined in halves ----
            hn = cpool.tile([P, N], F16, tag="hn")
            m0 = cpool.tile([P, N], F16, tag="m0")
            ex = cpool.tile([P, N], F16, tag="ex")
            exm1 = cpool.tile([P, N], F16, tag="exm1")
            ov = cpool.tile([P, NT * FOUT], F16, tag="ov")
            for qtr in range(4):
                for q in range(qtr * 4, qtr * 4 + 4):
                    rsq = cpool.tile([P, 1], F32, tag=f"rs{q}")
                    nc.vector.reciprocal(rsq[:], acc_ap(q, 128, 129))
                    if q % 2 == 0:
                        nc.scalar.activation(hn[:, q * P:(q + 1) * P],
                                             acc_ap(q, 0, 128), AF.Identity,
                                             scale=rsq[:])
                    else:
                        nc.vector.tensor_scalar(hn[:, q * P:(q + 1) * P],
                                                acc_ap(q, 0, 128), rsq[:],
                                                None, op0=ALU.mult)
                hs = slice(qtr * 512, qtr * 512 + 512)
                nc.vector.tensor_scalar(m0[:, hs], hn[:, hs], 0.0, None,
                                        op0=ALU.min)
                nc.scalar.activation(ex[:, hs], m0[:, hs], AF.Exp)
                nc.vector.tensor_scalar(exm1[:, hs], ex[:, hs], 1.0, None,
                                        op0=ALU.subtract)
                nc.vector.tensor_tensor(ov[:, hs], exm1[:, hs], hn[:, hs],
                                        ALU.max)
                if qtr % 2 == 1:
                    dhs = slice((qtr - 1) * 512, qtr * 512 + 512)
                    nc.scalar.dma_start(out_d[:, dhs], ov[:, dhs])
            acc_ctx.close()

    nc.compile()
    return nc


def make_in_maps(input, adj, W, b, a):
    x = np.asarray(input, dtype=np.float32)
    adj_np = np.asarray(adj)
    W_np = np.asarray(W, dtype=np.float32)
    b_np = np.asarray(b, dtype=np.float32)
    a_np = np.asarray(a, dtype=np.float32)
    a1 = a_np[:FOUT, 0]
    a2 = a_np[FOUT:, 0]

    # score-range calibration (cheap host dot products, sets fp16 shifts)
    w1 = W_np.T @ a1
    w2 = W_np.T @ a2
    s1 = x @ w1 + float(b_np @ a1)        # [B, N]
    s2 = x @ w2 + float(b_np @ a2)
    c1 = np.float32(s1.max() - MARGIN)
    c2 = np.float32(s2.max() - MARGIN)
    C = np.float32(c1 + c2)
    c1p = np.float32(ALPHA) * c1
    c2p = np.float32(C - c1p)
    cvecs = np.zeros((P, 8), dtype=np.float32)
    cvecs[:, 0] = -c1
    cvecs[:, 1] = -c1p
    cvecs[:, 2] = -c2
    cvecs[:, 3] = -c2p
    cvecs[:, 4] = -C

    # xT partition-contiguous: xTp[p, h*N + i] = x[i, h*128 + p]
    xT = x.transpose(0, 2, 1).reshape(B, 2, P, N)           # [B, h, p, i]
    xTp = np.ascontiguousarray(xT.transpose(0, 2, 1, 3)     # [B, p, h, i]
                               .reshape(B, P, 2 * N)).astype(np.float16)

    # packed small params: [wt0 | wt1 | a1rep | brow-rep | a2col | pad | cvecs]
    wt = W_np.T.astype(np.float16)                          # [256, 128]
    params = np.zeros((P, 640), dtype=np.float16)
    params[:, 0:128] = wt[0:P]
    params[:, 128:256] = wt[P:FIN]
    params[:, 256:384] = np.broadcast_to(a1[:, None], (FOUT, P)).astype(np.float16)
    params[:, 384:512] = np.broadcast_to(b_np[None, :], (P, FOUT)).astype(np.float16)
    params[:, 512] = a2.astype(np.float16)
    params[:, 624:640] = cvecs.view(np.float16)

    # mask [B, j, i] per-tile-type encoding, then partition-contiguous
    # groups: madjp[g*128 + p, (t%4)*N + i] = enc(adj[i, (4g + t%4)*128 + p])
    adjT = adj_np.transpose(0, 2, 1)       # [B, j, i]
    madj = np.empty((B, N, N), dtype=np.float16)
    alpha_rows = np.zeros(N, dtype=bool)
    for t in ALPHA_TILES:
        alpha_rows[t * P:(t + 1) * P] = True
    madj[:, alpha_rows, :] = np.where(
        adjT[:, alpha_rows, :] > 0, 0.0, MASK_NEG).astype(np.float16)
    madj[:, ~alpha_rows, :] = (adjT[:, ~alpha_rows, :] > 0).astype(np.float16)
    mg = madj.reshape(B, NG, GT, P, N)                       # [B, g, tg, p, i]
    madjp = np.ascontiguousarray(mg.transpose(0, 1, 3, 2, 4)  # [B, g, p, tg, i]
                                 .reshape(B, NG * P, GT * N))

    return [{"xTp": xTp[c], "madjp": madjp[c], "params": params}
            for c in range(B)]


def kernel(input, adj, W, b, a):
    from concourse.bass_utils import run_bass_kernel_spmd

    if "nc" not in _cache:
        _cache["nc"] = _build()
    nc = _cache["nc"]

    in_maps = make_in_maps(input, adj, W, b, a)
    res = run_bass_kernel_spmd(nc, in_maps, core_ids=list(range(B)))
    # outp[p, q*128 + o] -> out[q*128 + p, o]
    out = np.stack([
        np.asarray(res.results[c]["outp"]).reshape(P, NT, FOUT)
        .transpose(1, 0, 2).reshape(N, FOUT)
        for c in range(B)
    ])
    return np.ascontiguousarray(out.astype(np.float32))


# revision 49
# speedup vs baseline: 1.3610x; 1.1729x over previous
"""Batched GAT layer (B=8, N=2048, Fin=256, Fout=128) on 8 Trainium2 NeuronCores.

Data-parallel over batch B — one batch element per core. Per core, a
factored-exponential formulation keeps the O(N^2) element-wise work on the
DVE fp16 fast paths and the softmax contraction on the PE with the
denominator fused in as a 129th output column:

  h        = x @ W.T + b                       (PE fp16, fp32 psum)
  s1[i]    = a1.h_i,  s2[j] = a2.h_j           (PE)
  exp(lrelu(s1+s2)) = max(e^{s1}e^{s2}, e^{.4 s1}e^{.4 s2})   (lrelu piecewise)
  u = e^{s1-c1}, u' = e^{.4 s1-c1'}            (ACT, replicated row tiles)
  v = e^{s2-c2}, v' = e^{.4 s2-c2'}            (ACT, per-partition columns)
  p[j,i]   = m[j,i] * max(u_i v_j, u'_i v'_j)  (DVE TS/TT; m = 0/1 mask)
  acc[i,:] = sum_j p[j,i] * [h_j | 1]          (PE, p-stationary, S = col 128)
  out      = elu(acc[:, :128] / acc[:, 128])   (DVE + ACT tail)

Some j-tiles (ALPHA_TILES) instead use the additive-mask path
(em -> Prelu -> Exp on ACT) to balance DVE/ACT load; the host encodes mask
rows per tile type (0/-60 additive vs 1/0 multiplicative, fp16). Shifts
c1,c2 (softmax-invariant) keep everything in fp16 range; calibrated
host-side from cheap score maxima, passed as per-partition bias vectors.

DMA: the mask (8MB fp16) is host-relayouted partition-contiguous in groups
of 4 j-tiles so each of the 4 DMAs moves 16KB-contiguous runs per
partition (big packets, hardware-dynamic queue) instead of 4KB rows.
"""
import numpy as np

B, N, FIN, FOUT = 8, 2048, 256, 128
P = 128
NT = N // P           # 16 n-tiles
NC4 = N // 512        # 4 chunks of 512
NG = 4                # mask DMA groups
GT = NT // NG         # tiles per group (16KB per-partition runs)
ALPHA = 0.4
MASK_NEG = -60.0
MARGIN = 5.2

# j-tiles on the ACT additive-mask path (rest: factored DVE path).
# Spread across DMA groups so both engines stream smoothly.
ALPHA_TILES = frozenset({0, 2, 4, 6, 8, 10, 12, 14})
# f-tiles whose max/mask ops run on the Pool engine (early-mask tiles with
# slack); their matmuls are emitted later in the PE queue
POOL_TILES = (3, 5, 7)

_cache = {}


def _build():
    import concourse.mybir as mybir
    import concourse.tile as tile
    from concourse import bacc
    from concourse.masks import make_identity
    from contextlib import ExitStack

    F32 = mybir.dt.float32
    F16 = mybir.dt.float16
    AF = mybir.ActivationFunctionType
    ALU = mybir.AluOpType

    nc = bacc.Bacc("TRN2", target_bir_lowering=False, debug=False)

    # xT relayouted [128, 2, 2048] partition-contiguous; mask in 4 groups of
    # 4 j-tiles, each [128, 4, 2048] partition-contiguous; out written
    # [128, 16, 128] partition-major (host de-permutes).
    # xT chunked: [p, c*1024 + h*512 + i] = x[c*512 + i, h*128 + p]
    xT_d = nc.dram_tensor("xTp", [P, 2 * N], F16, kind="ExternalInput").ap()
    madj_d = nc.dram_tensor("madjp", [NG * P, GT * N], F16, kind="ExternalInput").ap()
    # all small params packed partition-contiguous into one tensor:
    # [wt0 | wt1 | a1rep | brow-rep | a2col | pad | cvecs-bytes]
    params_d = nc.dram_tensor("params", [P, 640], F16, kind="ExternalInput").ap()
    out_d = nc.dram_tensor("outp", [P, NT * FOUT], F16, kind="ExternalOutput").ap()

    with tile.TileContext(nc) as tc:
        with tc.tile_pool(name="const", bufs=1) as cpool, \
             tc.tile_pool(name="work", bufs=2) as wpool:
            # ---- packed params on the scalar queue; xT chunked on sync ----
            params = cpool.tile([P, 640], F16, tag="params")
            nc.scalar.dma_start(params[:], params_d)
            wt0 = params[:, 0:128]
            wt1 = params[:, 128:256]
            a1rep = params[:, 256:384]
            brow = params[0:1, 384:512]
            a2col = params[:, 512:513]
            cvecs = params[:, 624:640].bitcast(F32)
            # xT whole (8KB runs) first on the sync queue
            xt_all = cpool.tile([P, 2 * N], F16, tag="xt_all")
            nc.sync.dma_start(xt_all[:], xT_d)

            def xt0(c):
                return xt_all[:, c * 512:(c + 1) * 512]

            def xt1(c):
                return xt_all[:, N + c * 512:N + (c + 1) * 512]

            # mask: 4 groups of 4 tiles (16KB per-partition runs — big
            # packets keep the descriptor rate off the critical path),
            # alternating scalar/sync queues in loop-consumption order.
            # gpsimd issues nothing so the Pool engine is free for compute.
            adjm_all = cpool.tile([P, NT * N], F16, tag="adjm_all")
            for g in range(NG):
                eng = nc.scalar if g == 0 else nc.sync
                eng.dma_start(
                    adjm_all[:, g * GT * N:(g + 1) * GT * N],
                    madj_d[g * P:(g + 1) * P, :])

            def adjm(t):
                return adjm_all[:, t * N:(t + 1) * N]

            # constants built on gpsimd
            ident = cpool.tile([P, P], F16, tag="ident")
            make_identity(nc, ident[:])
            ones512 = cpool.tile([1, 512], F16, tag="ones512")
            nc.gpsimd.memset(ones512[:], 1.0)
            zcol = cpool.tile([1, P], F16, tag="zcol")
            nc.gpsimd.memset(zcol[:], 0.0)
            h_aug = []
            for t in range(NT):
                ha = cpool.tile([P, P + 4], F16, tag=f"haug{t}")
                nc.gpsimd.memset(ha[:, P:P + 1], 1.0)
                h_aug.append(ha)

            prep_ctx = ExitStack()
            pst = prep_ctx.enter_context(tc.tile_pool(name="pst", bufs=2, space="PSUM"))

            # ---- hT[o, n] = W x + b (bias via rank-1 matmul) ----
            hT = cpool.tile([FOUT, N], F16, tag="hT")
            for c in range(NC4):
                sl = slice(c * 512, (c + 1) * 512)
                hps = pst.tile([FOUT, 512], F32, tag="hps")
                nc.tensor.matmul(hps[:], wt0[:], xt0(c), start=True, stop=False)
                nc.tensor.matmul(hps[:], wt1[:], xt1(c), start=False, stop=False)
                nc.tensor.matmul(hps[:], brow[:], ones512[:], start=False, stop=True)
                nc.scalar.activation(hT[:, sl], hps[:], AF.Identity)

            # ---- s1b[p, i] = a1 . h_i (replicated across partitions) ----
            s1b = cpool.tile([P, N], F16, tag="s1b")
            for c in range(NC4):
                sl = slice(c * 512, (c + 1) * 512)
                bps = pst.tile([P, 512], F32, tag="bps")
                nc.tensor.matmul(bps[:], a1rep[:], hT[:, sl], start=True, stop=True)
                nc.scalar.activation(s1b[:, sl], bps[:], AF.Identity)

            # ---- s2 columns: s2cols[p, t] = a2 . h_{t*128+p} ----
            s2ps = pst.tile([P, NT], F32, tag="s2ps")
            for t in range(NT):
                nc.tensor.matmul(s2ps[:, t:t + 1], hT[:, t * P:(t + 1) * P],
                                 a2col[:], start=True, stop=True,
                                 skip_group_check=True)
            s2cols = cpool.tile([P, NT], F32, tag="s2cols")
            nc.vector.tensor_copy(s2cols[:], s2ps[:])

            # ---- v / v' columns (fp32) and u / u' reps (fp16) ----
            vcols = cpool.tile([P, NT], F32, tag="vcols")
            nc.scalar.activation(vcols[:], s2cols[:], AF.Exp, bias=cvecs[:, 2:3])
            vpcols = cpool.tile([P, NT], F32, tag="vpcols")
            nc.scalar.activation(vpcols[:], s2cols[:], AF.Exp, bias=cvecs[:, 3:4],
                                 scale=ALPHA)
            urep = cpool.tile([P, N], F16, tag="urep")
            nc.scalar.activation(urep[:], s1b[:], AF.Exp, bias=cvecs[:, 0:1])
            uprep = cpool.tile([P, N], F16, tag="uprep")
            nc.scalar.activation(uprep[:], s1b[:], AF.Exp, bias=cvecs[:, 1:2],
                                 scale=ALPHA)

            # ---- h_aug tiles via PE transpose of hT ----
            for t in range(NT):
                tps = pst.tile([P, P], F16, tag="tps")
                nc.tensor.transpose(tps[:], hT[:, t * P:(t + 1) * P], ident[:])
                nc.vector.tensor_copy(h_aug[t][:, 0:P], tps[:])

            prep_ctx.close()

            # ---- PSUM accumulators: 16 slices packed 3-per-bank ----
            acc_ctx = ExitStack()
            psacc = acc_ctx.enter_context(
                tc.tile_pool(name="psacc", bufs=1, space="PSUM"))
            accb = [psacc.tile([P, 512], F32, tag=f"accb{k}", name=f"accb{k}")
                    for k in range(6)]
            # start+stop zero-fill each bank once; the 16 accumulation slices
            # then run accumulate-only (multiple open start-groups per bank
            # lose their staged first write).
            for k in range(6):
                nc.tensor.matmul(accb[k][:], zcol[:], ones512[:],
                                 start=True, stop=True, skip_group_check=True)

            def acc_ap(q, lo, hi):
                base = (q % 3) * 160
                return accb[q // 3][:, base + lo:base + hi]

            # ---- main loop over j-tiles ----
            def emit_mms(t, pt, last=False):
                for q in range(NT):
                    nc.tensor.matmul(acc_ap(q, 0, 129), pt[:, q * P:(q + 1) * P],
                                     h_aug[t][:, 0:129], start=False, stop=last,
                                     skip_group_check=True)

            defer_after = {}
            deferred_pt = {}
            for t in range(NT):
                if t in ALPHA_TILES:
                    em = wpool.tile([P, N], F16, tag="em", bufs=3)
                    nc.vector.tensor_tensor(em[:], adjm(t), s1b[:], ALU.add)
                    lt = wpool.tile([P, N], F16, tag="lt")
                    nc.scalar.activation(lt[:], em[:], AF.Prelu,
                                         bias=s2cols[:, t:t + 1],
                                         scale=1.0, alpha=ALPHA)
                    pt = wpool.tile([P, N], F16, tag="pt", bufs=8)
                    nc.scalar.activation(pt[:], lt[:], AF.Exp, bias=cvecs[:, 4:5])
                else:
                    veng = nc.vector
                    t1a = wpool.tile([P, N], F16, tag="t1a")
                    nc.vector.tensor_scalar(t1a[:], urep[:], vcols[:, t:t + 1],
                                            None, op0=ALU.mult)
                    t2 = wpool.tile([P, N], F16, tag="t2")
                    nc.vector.tensor_scalar(t2[:], uprep[:], vpcols[:, t:t + 1],
                                            None, op0=ALU.mult)
                    mx = wpool.tile([P, N], F16, tag="lt")
                    nc.vector.tensor_tensor(mx[:], t1a[:], t2[:], ALU.max)
                    pt = wpool.tile([P, N], F16, tag="pt", bufs=8)
                    veng.tensor_tensor(pt[:], mx[:], adjm(t), ALU.mult)

                if defer_after and t in POOL_TILES:
                    deferred_pt[t] = pt
                else:
                    emit_mms(t, pt, last=(t == NT - 1))
                if t in defer_after:
                    dt = defer_after[t]
                    emit_mms(dt, deferred_pt.pop(dt))

            # ---- tail: normalize + elu, pipelined in halves ----
            hn = cpool.tile([P, N], F16, tag="hn")
            m0 = cpool.tile([P, N], F16, tag="m0")
            ex = cpool.tile([P, N], F16, tag="ex")
            exm1 = cpool.tile([P, N], F16, tag="exm1")
            ov = cpool.tile([P, NT * FOUT], F16, tag="ov")
            for half in range(2):
                for q in range(half * 8, half * 8 + 8):
                    rsq = cpool.tile([P, 1], F32, tag=f"rs{q}")
                    nc.vector.reciprocal(rsq[:], acc_ap(q, 128, 129))
                    if q % 2 == 0:
                        nc.scalar.activation(hn[:, q * P:(q + 1) * P],
                                             acc_ap(q, 0, 128), AF.Identity,
                                             scale=rsq[:])
                    else:
                        nc.vector.tensor_scalar(hn[:, q * P:(q + 1) * P],
                                                acc_ap(q, 0, 128), rsq[:],
                                                None, op0=ALU.mult)
                hs = slice(half * 1024, half * 1024 + 1024)
                nc.vector.tensor_scalar(m0[:, hs], hn[:, hs], 0.0, None,
                                        op0=ALU.min)
                nc.scalar.activation(ex[:, hs], m0[:, hs], AF.Exp)
                nc.vector.tensor_scalar(exm1[:, hs], ex[:, hs], 1.0, None,
                                        op0=ALU.subtract)
                nc.vector.tensor_tensor(ov[:, hs], exm1[:, hs], hn[:, hs],
                                        ALU.max)
                nc.scalar.dma_start(out_d[:, hs], ov[:, hs])
            acc_ctx.close()

    nc.compile()
    return nc


def make_in_maps(input, adj, W, b, a):
    x = np.asarray(input, dtype=np.float32)
    adj_np = np.asarray(adj)
    W_np = np.asarray(W, dtype=np.float32)
    b_np = np.asarray(b, dtype=np.float32)
    a_np = np.asarray(a, dtype=np.float32)
    a1 = a_np[:FOUT, 0]
    a2 = a_np[FOUT:, 0]

    # score-range calibration (cheap host dot products, sets fp16 shifts)
    w1 = W_np.T @ a1
    w2 = W_np.T @ a2
    s1 = x @ w1 + float(b_np @ a1)        # [B, N]
    s2 = x @ w2 + float(b_np @ a2)
    c1 = np.float32(s1.max() - MARGIN)
    c2 = np.float32(s2.max() - MARGIN)
    C = np.float32(c1 + c2)
    c1p = np.float32(ALPHA) * c1
    c2p = np.float32(C - c1p)
    cvecs = np.zeros((P, 8), dtype=np.float32)
    cvecs[:, 0] = -c1
    cvecs[:, 1] = -c1p
    cvecs[:, 2] = -c2
    cvecs[:, 3] = -c2p
    cvecs[:, 4] = -C

    # xT partition-contiguous: xTp[p, h*N + i] = x[i, h*128 + p]
    xT = x.transpose(0, 2, 1).reshape(B, 2, P, N)           # [B, h, p, i]
    xTp = np.ascontiguousarray(xT.transpose(0, 2, 1, 3)     # [B, p, h, i]
                               .reshape(B, P, 2 * N)).astype(np.float16)

    # packed small params: [wt0 | wt1 | a1rep | brow-rep | a2col | pad | cvecs]
    wt = W_np.T.astype(np.float16)                          # [256, 128]
    params = np.zeros((P, 640), dtype=np.float16)
    params[:, 0:128] = wt[0:P]
    params[:, 128:256] = wt[P:FIN]
    params[:, 256:384] = np.broadcast_to(a1[:, None], (FOUT, P)).astype(np.float16)
    params[:, 384:512] = np.broadcast_to(b_np[None, :], (P, FOUT)).astype(np.float16)
    params[:, 512] = a2.astype(np.float16)
    params[:, 624:640] = cvecs.view(np.float16)

    # mask [B, j, i] per-tile-type encoding, then partition-contiguous
    # groups: madjp[g*128 + p, (t%4)*N + i] = enc(adj[i, (4g + t%4)*128 + p])
    adjT = adj_np.transpose(0, 2, 1)       # [B, j, i]
    madj = np.empty((B, N, N), dtype=np.float16)
    alpha_rows = np.zeros(N, dtype=bool)
    for t in ALPHA_TILES:
        alpha_rows[t * P:(t + 1) * P] = True
    madj[:, alpha_rows, :] = np.where(
        adjT[:, alpha_rows, :] > 0, 0.0, MASK_NEG).astype(np.float16)
    madj[:, ~alpha_rows, :] = (adjT[:, ~alpha_rows, :] > 0).astype(np.float16)
    mg = madj.reshape(B, NG, GT, P, N)                       # [B, g, tg, p, i]
    madjp = np.ascontiguousarray(mg.transpose(0, 1, 3, 2, 4)  # [B, g, p, tg, i]
                                 .reshape(B, NG * P, GT * N))

    return [{"xTp": xTp[c], "madjp": madjp[c], "params": params}
            for c in range(B)]


def kernel(input, adj, W, b, a):
    from concourse.bass_utils import run_bass_kernel_spmd

    if "nc" not in _cache:
        _cache["nc"] = _build()
    nc = _cache["nc"]

    in_maps = make_in_maps(input, adj, W, b, a)
    res = run_bass_kernel_spmd(nc, in_maps, core_ids=list(range(B)))
    # outp[p, q*128 + o] -> out[q*128 + p, o]
    out = np.stack([
        np.asarray(res.results[c]["outp"]).reshape(P, NT, FOUT)
        .transpose(1, 0, 2).reshape(N, FOUT)
        for c in range(B)
    ])
    return np.ascontiguousarray(out.astype(np.float32))
